# revision 6
# baseline (speedup 1.0000x reference)
"""MultiHeadGraphAttention Trainium2 kernel (v5: ACT offload + DMA fixes).

Data-parallel over batch: core b computes batch element b (B=8, 8 cores).

Per-core math (one batch element, N=2048 nodes, U=256 units, H=8 heads, d=32):
  Q = x Wq, K = x Wk, V = x Wv
  sT[k,q]  = sum_d KT[d,k] QT[d,q]           (scores, transposed layout)
  e        = exp(sT/sqrt(d)) * adjT          (masked exp)
  ctxT[d,q] = sum_k V[k,d] e[k,q] ; Z[q] = sum_k e[k,q]
  out      = (ctxT/Z).T @ Wo + bo

v5 structure (baseline was ACT-bound at ~1107ns/pair steady):
  - Three masked-exp modes per head pair, rotated per (qc,kb) as
    (S, G, A, A) to balance ACT / DVE / GPSIMD:
      'A': ACT exp + DVE bf16 mask multiply        (ACT 1147 + DVE 690)
      'S': ONE DVE scalar_tensor_tensor:
           i16 = score*SCH_A + mb;  bitcast bf16 ~ masked exp (DVE 1223)
      'G': ACT exp + GPSIMD mask multiply (2x [128,512]); PV deferred one
           iteration so the slow GPSIMD mask never stalls the PE queue.
  - mb[k,q] = adj ? 16224 : 2000 (i16). Masked entries: i16 = A*s+2000
    in [800,3200] -> bf16 ~2e-31 ~ 0. Unmasked: bf16 bits of
    16224+A*s ~ 0.875*exp(s*scale) (2^(-32/128)*E[(1+f)/2^f] = 0.875).
    The SAME tensor bitcast to bf16 is the multiplicative mask for
    A/G pairs: {0, 0.875} -- matching means, so the 0.875 cancels in
    softmax normalization.
  - Normalize chain: reciprocal directly on the cps PSUM Z rows
    (partition-stride-64 AP) -> [2,512] SBUF -> ONE broadcast DMA per
    tile -> zinv rows -> ctxn multiply (shorter qc-boundary chain).
  - Mask DMAs ride the GPSIMD SWDGE ring (parallel with sync-ring
    xT/weight loads); output is relayouted [128, 16*256] with per-qc
    merged DMAs (4KB/partition contiguous) to compress the tail.
  - Z is FUSED into the PV matmul: stationary [ones | V_h | zeros31]
    (M=64) makes row 0 of each 64-row block the softmax denominator.
  - wo4: Wo rows permuted into per-pair-tile chunks matching ctx layout.
"""

import sys

for p in ("/opt/trn_rl_repo",):
    if p not in sys.path:
        sys.path.insert(0, p)

from contextlib import ExitStack

import numpy as np
import ml_dtypes

import concourse.bass as bass
import concourse.mybir as mybir
import concourse.tile as tile
from concourse import bacc
from concourse.bass_utils import run_bass_kernel_spmd

B, N, U, H, D = 8, 2048, 256, 8, 32
NB = N // 128          # 16 key blocks of 128
QC = 4                 # q chunks
QW = N // QC           # 512 q per chunk
SCALE = 1.0 / np.sqrt(np.float32(D))
# Schraudolph bf16: bits(bf16(y)) ~= 16256 + 128*log2(y)
SCH_A = 128.0 * float(SCALE) * 1.4426950408889634
MB_VAL = 16224         # additive i16 bias (unmasked); bitcast bf16 = 0.875
MB_MASKED = 2000       # masked bias: codes stay in [800,3200] -> bf16 ~2e-31
                       # (bias 0 would let A*s land in 0xFF81..0xFFFF = NaN)

f32 = mybir.dt.float32
bf16 = mybir.dt.bfloat16
i16 = mybir.dt.int16
EXP = mybir.ActivationFunctionType.Exp
MULT = mybir.AluOpType.mult
ADD = mybir.AluOpType.add


def pair_mode(qc, kb, p):
    """Masked-exp engine path for head pair p of (qc,kb).

    Pair 0 -> 'S' (DVE-only fused Schraudolph+mask).
    Pair 1 -> 'G' (ACT exp + GPSIMD mask), except: kb=15 (normalize
      deadline -- G's PV lags one iteration) and qc=0,kb<3 (gpsimd queue
      is still emitting the mask SWDGE DMAs).
    Pairs 2,3 -> 'A' (ACT exp + DVE mask).
    """
    if p == 0:
        return "S"
    if p == 1:
        if kb >= NB - 1 or (qc == 0 and kb < 3):
            return "A"
        return "G"
    return "A"


def build_program():
    nc = bacc.Bacc("TRN2", target_bir_lowering=False, debug=False,
                   enable_asserts=False, num_devices=B)

    xT_d = nc.dram_tensor("xT", [U, N], bf16, kind="ExternalInput").ap()
    mb_d = nc.dram_tensor("mb", [N, N], i16, kind="ExternalInput").ap()
    wq_d = nc.dram_tensor("Wq", [U, U], bf16, kind="ExternalInput").ap()
    wk_d = nc.dram_tensor("Wk", [U, U], bf16, kind="ExternalInput").ap()
    wv_d = nc.dram_tensor("Wv", [U, U], bf16, kind="ExternalInput").ap()
    wo4_d = nc.dram_tensor("wo4", [4 * 128, U], bf16, kind="ExternalInput").ap()
    bo_d = nc.dram_tensor("bo", [U], f32, kind="ExternalInput").ap()
    out_d = nc.dram_tensor("out", [128, NB * U], f32, kind="ExternalOutput").ap()

    with tile.TileContext(nc) as tc:
        with ExitStack() as ctx:
            kernel_body(ctx, tc, xT_d, mb_d, wq_d, wk_d, wv_d, wo4_d,
                        bo_d, out_d)
    nc.compile()
    return nc


def kernel_body(ctx, tc, xT_d, mb_d, wq_d, wk_d, wv_d, wo4_d, bo_d, out_d):
    nc = tc.nc
    persist = ctx.enter_context(tc.tile_pool(name="persist", bufs=1))
    stage = ctx.enter_context(tc.tile_pool(name="stage", bufs=2))
    epool = ctx.enter_context(tc.tile_pool(name="epool", bufs=10))
    espool = ctx.enter_context(tc.tile_pool(name="espool", bufs=3))
    zpool = ctx.enter_context(tc.tile_pool(name="zpool", bufs=4))
    spool = ctx.enter_context(tc.tile_pool(name="spool", bufs=2, space="PSUM"))
    cpool = ctx.enter_context(tc.tile_pool(name="cpool", bufs=4, space="PSUM"))

    # ---- input DMAs: x and weights on the sync HWDGE ring (they gate the
    # projections); the 8MB mask on the GPSIMD SWDGE ring in parallel,
    # except kb 0-2 which are needed almost immediately ------------------
    xT = [stage.tile([128, N], bf16, tag="stage", name=f"xT{c}") for c in range(2)]
    for c in range(2):
        nc.sync.dma_start(xT[c][:], xT_d[c * 128:(c + 1) * 128, :])
    w_sb = {}
    for nm, dram in (("wq", wq_d), ("wk", wk_d), ("wv", wv_d)):
        w_sb[nm] = persist.tile([128, 2 * U], bf16, tag=nm, name=nm)
        for c in range(2):
            nc.sync.dma_start(w_sb[nm][:, c * U:(c + 1) * U],
                              dram[c * 128:(c + 1) * 128, :])
    wo_sb = persist.tile([128, 4 * U], bf16, tag="wo4")
    for t in range(4):
        nc.sync.dma_start(wo_sb[:, t * U:(t + 1) * U],
                          wo4_d[t * 128:(t + 1) * 128, :])
    bo_sb = persist.tile([1, U], f32, tag="bo")
    nc.sync.dma_start(bo_sb[:], bo_d.rearrange("(o n) -> o n", o=1))
    bo_bc = persist.tile([128, U], f32, tag="bo_bc")
    nc.sync.dma_start(bo_bc[:],
                      bo_sb[:].unsqueeze(1).broadcast_to([1, 128, U]))

    # mask: i16 additive-bias form; bf16 view is the multiplicative mask
    mb_sb = persist.tile([128, NB * N], i16, tag="mb")
    mbf_sb = mb_sb.bitcast(bf16)
    for kb in range(3):
        nc.sync.dma_start(mb_sb[:, kb * N:(kb + 1) * N],
                          mb_d[kb * 128:(kb + 1) * 128, :])
    for kb in range(3, NB):
        nc.gpsimd.dma_start(mb_sb[:, kb * N:(kb + 1) * N],
                            mb_d[kb * 128:(kb + 1) * 128, :])

    # ---- persistent SBUF tensors -------------------------------------------
    qT = [persist.tile([128, N], bf16, tag=f"qT{c}", name=f"qT{c}") for c in range(2)]
    kT = [persist.tile([128, N], bf16, tag=f"kT{c}", name=f"kT{c}") for c in range(2)]
    # augmented V: per (kb, h) a 64-col block [ones | V_h (32) | zeros31]
    v_aug = persist.tile([128, NB * H * 64], bf16, tag="vaug")
    nc.vector.memset(v_aug[:], 0.0)
    nc.vector.memset(
        v_aug.rearrange("p (b c) -> p b c", c=64)[:, :, 0:1], 1.0)
    # normalized context per pair tile t: rows 1-32 head-lo, 65-96 head-hi
    ctxn = [persist.tile([128, N], bf16, tag=f"ctxn{t}", name=f"ctxn{t}")
            for t in range(4)]
    out_sb = persist.tile([128, NB * U], f32, tag="out_sb")
    # 1/Z broadcast target: rows 1-32 / 65-96 per tile column-range get the
    # reciprocal; all other rows stay 1.0 forever (junk cps rows are 0.0,
    # and 0*finite=0 keeps ctxn junk rows NaN-free for the wo4 zero rows).
    zinv4 = persist.tile([128, 4 * QW], f32, tag="zinv4")
    nc.vector.memset(zinv4[:], 1.0)

    # ---- projections (bf16 operands) ---------------------------------------
    for nn in range(2):
        for g in range(2):
            for w, dst in (("wq", qT), ("wk", kT)):
                ps = spool.tile([128, 2 * QW], f32, tag="s", name="projps")
                for half in range(2):
                    sl = slice(half * QW, (half + 1) * QW)
                    tok = slice(nn * 2 * QW + half * QW,
                                nn * 2 * QW + (half + 1) * QW)
                    for kc in range(2):
                        nc.tensor.matmul(
                            ps[:, sl],
                            w_sb[w][:, (kc * 2 + g) * 128:(kc * 2 + g + 1) * 128],
                            xT[kc][:, tok],
                            start=(kc == 0), stop=(kc == 1))
                if w == "wq":   # split prologue copies across ACT and DVE
                    nc.scalar.copy(dst[g][:, nn * 2 * QW:(nn + 1) * 2 * QW], ps[:])
                else:
                    nc.vector.tensor_copy(dst[g][:, nn * 2 * QW:(nn + 1) * 2 * QW], ps[:])
    # V = x @ Wv -> strided into v_aug (head h of block kb at col 64h+1)
    for kb in range(NB):
        ps = spool.tile([128, U], f32, tag="s", name="vps")
        for kc in range(2):
            nc.tensor.matmul(
                ps[:],
                xT[kc][:, kb * 128:(kb + 1) * 128],
                w_sb["wv"][:, kc * U:(kc + 1) * U],
                start=(kc == 0), stop=(kc == 1))
        nc.vector.tensor_copy(
            v_aug.rearrange("p (b c) -> p b c", c=64)
            [:, kb * H:(kb + 1) * H, 1:1 + D],
            ps.rearrange("p (h d) -> p h d", d=D))

    # ---- main attention loop, one-step PE software pipeline -----------------
    # pair p = 2g+pi covers heads 4g+pi, 4g+pi+2 (j = pi, pi+2)
    cps = {}

    def emit_scores_pair(qc, kb, p):
        qs = qc * QW
        g, pi = p // 2, p % 2
        sps = spool.tile([128, 2 * QW], f32, tag="s", name=f"sps{qc}_{kb}_{p}")
        for jj in range(2):
            j = pi + 2 * jj
            nc.tensor.matmul(
                sps[:, jj * QW:(jj + 1) * QW],
                kT[g][32 * j:32 * (j + 1), kb * 128:(kb + 1) * 128],
                qT[g][32 * j:32 * (j + 1), qs:qs + QW],
                start=True, stop=True,
                tile_position=(32 * j, 0))
        return sps

    def emit_exp(qc, kb, p, sps):
        """ACT/DVE/GPSIMD part of the masked exp; returns the e source AP."""
        qs = qc * QW
        mode = pair_mode(qc, kb, p)
        if mode == "S":
            es = espool.tile([128, 2 * QW], i16, tag="es",
                             name=f"es{qc}_{kb}_{p}")
            nc.vector.scalar_tensor_tensor(
                es.rearrange("p (j q) -> p j q", j=2),
                sps.rearrange("p (j q) -> p j q", j=2),
                float(SCH_A),
                mb_sb[:, kb * N + qs:kb * N + qs + QW]
                .unsqueeze(1).broadcast_to([128, 2, QW]),
                MULT, ADD)
            return es.bitcast(bf16)
        e = epool.tile([128, 2 * QW], bf16, tag="e", name=f"e{qc}_{kb}_{p}")
        nc.scalar.activation(e[:], sps[:], EXP, scale=float(SCALE))
        me = mbf_sb[:, kb * N + qs:kb * N + qs + QW]
        if mode == "A":
            nc.vector.tensor_tensor(
                e.rearrange("p (j q) -> p j q", j=2),
                e.rearrange("p (j q) -> p j q", j=2),
                me.unsqueeze(1).broadcast_to([128, 2, QW]), MULT)
        else:  # "G": two plain [128,512] gpsimd multiplies (no broadcast AP)
            for jj in range(2):
                sl = slice(jj * QW, (jj + 1) * QW)
                nc.gpsimd.tensor_tensor(e[:, sl], e[:, sl], me, MULT)
        return e

    def emit_pv(qc, kb, p, e):
        g, pi = p // 2, p % 2
        for jj in range(2):
            h = 4 * g + pi + 2 * jj
            ej = e[:, jj * QW:(jj + 1) * QW]
            nc.tensor.matmul(
                cps[qc, p][64 * jj:64 * jj + 64, :],
                v_aug[:, (kb * H + h) * 64:(kb * H + h + 1) * 64],
                ej, start=(kb == 0), stop=(kb == NB - 1),
                tile_position=(0, 64 * jj))

    def emit_normalize_tile(qc, t):
        # reciprocal over the whole cps tile (DVE time is free-dim-bound;
        # junk rows give 1/0=Inf but only rows 0/64 are ever read), then
        # broadcast DMAs into zinv rows 1-32 / 65-96 of tile t's column
        # range, then the normalize multiply (frees cps[qc,t]).
        qs = qc * QW
        zf = zpool.tile([128, QW], f32, tag="zf", name=f"zf{qc}_{t}")
        nc.vector.reciprocal_approx_fast(zf[:], cps[qc, t][:])
        for g in range(2):
            nc.sync.dma_start(
                zinv4[64 * g + 1:64 * g + 33, t * QW:(t + 1) * QW],
                zf[64 * g:64 * g + 1, :]
                .unsqueeze(1).broadcast_to([1, 32, QW]))
        nc.vector.tensor_tensor(
            ctxn[t][:, qs:qs + QW], cps[qc, t][:],
            zinv4[:, t * QW:(t + 1) * QW], MULT)

    def emit_outproj_qb(qb, last=False):
        ops = spool.tile([128, U], f32, tag="s", name=f"ops{qb}")
        for t in range(4):
            nc.tensor.matmul(
                ops[:],
                ctxn[t][:, qb * 128:(qb + 1) * 128],
                wo_sb[:, t * U:(t + 1) * U],
                start=(t == 0), stop=(t == 3))
        nc.vector.tensor_tensor(out_sb[:, qb * U:(qb + 1) * U], ops[:],
                                bo_bc[:], ADD)
        if last:
            # merged 4-qb DMA: 4KB/partition contiguous, one InstDMACopy
            qc = qb // 4
            nc.sync.dma_start(
                out_d[:, qc * 4 * U:(qc + 1) * 4 * U],
                out_sb[:, qc * 4 * U:(qc + 1) * 4 * U])

    # software pipeline: scores for pairs (2t, 2t+1) back-to-back, then
    # exp/mask/PV for the previous 2-pair group. 'G' pairs' PVs are
    # deferred one extra iteration (gpsimd masks are slow); the deferral
    # queue preserves per-tile kb order, and kb=15 is never 'G'.
    prev = None
    gq = []                 # deferred (qc, kb, p, e) for 'G' pairs
    pending_outproj = []

    def handle_prev(pqc, pkb, pt, psp0, psp1):
        p0, p1 = 2 * pt, 2 * pt + 1
        e0 = emit_exp(pqc, pkb, p0, psp0)
        e1 = emit_exp(pqc, pkb, p1, psp1)
        m1 = pair_mode(pqc, pkb, p1)
        # drain one deferred G-PV first (its mask had a full iteration)
        if gq:
            emit_pv(*gq.pop(0))
        emit_pv(pqc, pkb, p0, e0)
        if m1 == "G":
            gq.append((pqc, pkb, p1, e1))
        else:
            emit_pv(pqc, pkb, p1, e1)
        if pkb == NB - 1:
            while gq:
                emit_pv(*gq.pop(0))
            emit_normalize_tile(pqc, 2 * pt)
            emit_normalize_tile(pqc, 2 * pt + 1)
            if pt == 1:
                pending_outproj.extend(pqc * QC + i for i in range(QC))
        elif (pending_outproj and pt == 1 and pkb in (1, 5, 9, 13)):
            qb = pending_outproj.pop(0)
            emit_outproj_qb(qb, last=(qb % 4 == 3))

    for qc in range(QC):
        for p in range(4):
            cps[qc, p] = cpool.tile([128, QW], f32, tag="c", name=f"cps{qc}_{p}")
        for kb in range(NB):
            for t in range(2):
                sp0 = emit_scores_pair(qc, kb, 2 * t)
                sp1 = emit_scores_pair(qc, kb, 2 * t + 1)
                if prev is not None:
                    handle_prev(*prev)
                prev = (qc, kb, t, sp0, sp1)
    handle_prev(*prev)
    for qb in pending_outproj:
        emit_outproj_qb(qb, last=(qb % 4 == 3))


_CACHED = None


def _get_program():
    global _CACHED
    if _CACHED is None:
        _CACHED = build_program()
    return _CACHED


def _bf16(a):
    return np.asarray(a, dtype=ml_dtypes.bfloat16)


def _build_wo4(Wo):
    """Permute Wo rows into 4 chunks matching the fused ctx layout.

    Pair tile t = 2g+pi: row r=1..32 -> Wo row for head 4g+pi dim r-1;
    row r=65..96 -> head 4g+pi+2 dim r-65; other rows zero.
    """
    wo4 = np.zeros((4 * 128, U), dtype=np.float32)
    Wo = np.asarray(Wo, np.float32)
    for g in range(2):
        for pi in range(2):
            t = 2 * g + pi
            for jj in range(2):
                h = 4 * g + pi + 2 * jj
                wo4[t * 128 + 64 * jj + 1:t * 128 + 64 * jj + 33, :] = \
                    Wo[h * D:(h + 1) * D, :]
    return _bf16(wo4)


def kernel(node_features, adjacency_matrix, Wq, Wk, Wv, Wo, bo, **run_kwargs):
    nc = _get_program()
    xT = _bf16(np.transpose(np.asarray(node_features, np.float32), (0, 2, 1)))
    adjT = np.transpose(np.asarray(adjacency_matrix), (0, 2, 1))
    mb = np.where(adjT > 0, np.int16(MB_VAL), np.int16(MB_MASKED))
    wo4 = _build_wo4(Wo)
    wq, wk, wv = _bf16(Wq), _bf16(Wk), _bf16(Wv)
    bo32 = np.asarray(bo, np.float32)
    in_maps = []
    for b in range(B):
        in_maps.append({
            "xT": np.ascontiguousarray(xT[b]),
            "mb": np.ascontiguousarray(mb[b]),
            "Wq": wq, "Wk": wk, "Wv": wv, "wo4": wo4,
            "bo": bo32,
        })
    res = run_bass_kernel_spmd(nc, in_maps, core_ids=list(range(B)), **run_kwargs)
    # out_sb layout: [p, qb*256+u] holds token qb*128+p
    out = np.stack(
        [res.results[b]["out"].reshape(128, NB, U).transpose(1, 0, 2)
         .reshape(N, U) for b in range(B)], axis=0)
    kernel.last_results = res
    return out


# revision 12
# speedup vs baseline: 1.2294x; 1.2294x over previous
"""MultiHeadGraphAttention Trainium2 kernel (v5: ACT offload + DMA fixes).

Data-parallel over batch: core b computes batch element b (B=8, 8 cores).

Per-core math (one batch element, N=2048 nodes, U=256 units, H=8 heads, d=32):
  Q = x Wq, K = x Wk, V = x Wv
  sT[k,q]  = sum_d KT[d,k] QT[d,q]           (scores, transposed layout)
  e        = exp(sT/sqrt(d)) * adjT          (masked exp)
  ctxT[d,q] = sum_k V[k,d] e[k,q] ; Z[q] = sum_k e[k,q]
  out      = (ctxT/Z).T @ Wo + bo

v5 structure (baseline was ACT-bound at ~1107ns/pair steady):
  - Three masked-exp modes per head pair, rotated per (qc,kb) as
    (S, G, A, A) to balance ACT / DVE / GPSIMD:
      'A': ACT exp + DVE bf16 mask multiply        (ACT 1147 + DVE 690)
      'S': ONE DVE scalar_tensor_tensor:
           i16 = score*SCH_A + mb;  bitcast bf16 ~ masked exp (DVE 1223)
      'G': ACT exp + GPSIMD mask multiply (2x [128,512]); PV deferred one
           iteration so the slow GPSIMD mask never stalls the PE queue.
  - mb[k,q] = adj ? 16224 : 2000 (i16). Masked entries: i16 = A*s+2000
    in [800,3200] -> bf16 ~2e-31 ~ 0. Unmasked: bf16 bits of
    16224+A*s ~ 0.875*exp(s*scale) (2^(-32/128)*E[(1+f)/2^f] = 0.875).
    The SAME tensor bitcast to bf16 is the multiplicative mask for
    A/G pairs: {0, 0.875} -- matching means, so the 0.875 cancels in
    softmax normalization.
  - Normalize chain: reciprocal directly on the cps PSUM Z rows
    (partition-stride-64 AP) -> [2,512] SBUF -> ONE broadcast DMA per
    tile -> zinv rows -> ctxn multiply (shorter qc-boundary chain).
  - Mask DMAs ride the GPSIMD SWDGE ring (parallel with sync-ring
    xT/weight loads); output is relayouted [128, 16*256] with per-qc
    merged DMAs (4KB/partition contiguous) to compress the tail.
  - Z is FUSED into the PV matmul: stationary [ones | V_h | zeros31]
    (M=64) makes row 0 of each 64-row block the softmax denominator.
  - wo4: Wo rows permuted into per-pair-tile chunks matching ctx layout.
"""

import sys

for p in ("/opt/trn_rl_repo",):
    if p not in sys.path:
        sys.path.insert(0, p)

from contextlib import ExitStack

import numpy as np
import ml_dtypes

import concourse.bass as bass
import concourse.mybir as mybir
import concourse.tile as tile
from concourse import bacc
from concourse.bass_utils import run_bass_kernel_spmd

B, N, U, H, D = 8, 2048, 256, 8, 32
NB = N // 128          # 16 key blocks of 128
QC = 4                 # q chunks
QW = N // QC           # 512 q per chunk
SCALE = 1.0 / np.sqrt(np.float32(D))
# Schraudolph bf16: bits(bf16(y)) ~= 16256 + 128*log2(y)
SCH_A = 128.0 * float(SCALE) * 1.4426950408889634
MB_VAL = 16224         # additive i16 bias (unmasked); bitcast bf16 = 0.875
MB_MASKED = 2000       # masked bias: codes stay in [800,3200] -> bf16 ~2e-31
                       # (bias 0 would let A*s land in 0xFF81..0xFFFF = NaN)

f32 = mybir.dt.float32
bf16 = mybir.dt.bfloat16
i16 = mybir.dt.int16
EXP = mybir.ActivationFunctionType.Exp
MULT = mybir.AluOpType.mult
ADD = mybir.AluOpType.add


def pair_mode(qc, kb, p):
    """Masked-exp engine path for head pair p of (qc,kb).

    Pair 0 -> 'S' (DVE-only fused Schraudolph+mask).
    Pairs 1-3 -> 'A' (ACT exp + DVE mask); pairs 2,3 share one e tile so
    their mask is a single wide DVE op (1223ns vs 2x690).
    v5 note: a 'G' mode (GPSIMD mask) was tried and abandoned -- the
    2.7-3.2us gpsimd mask chains stalled the in-order PE queue even with
    a one-iteration PV deferral (446us vs 367us baseline).
    """
    return "S" if p == 0 else "A"


def build_program():
    nc = bacc.Bacc("TRN2", target_bir_lowering=False, debug=False,
                   enable_asserts=False, num_devices=B)

    xT_d = nc.dram_tensor("xT", [U, N], bf16, kind="ExternalInput").ap()
    mb_d = nc.dram_tensor("mb", [N, N], i16, kind="ExternalInput").ap()
    wq_d = nc.dram_tensor("Wq", [U, U], bf16, kind="ExternalInput").ap()
    wk_d = nc.dram_tensor("Wk", [U, U], bf16, kind="ExternalInput").ap()
    wv_d = nc.dram_tensor("Wv", [U, U], bf16, kind="ExternalInput").ap()
    wo4_d = nc.dram_tensor("wo4", [4 * 128, U], bf16, kind="ExternalInput").ap()
    bo_d = nc.dram_tensor("bo", [U], f32, kind="ExternalInput").ap()
    out_d = nc.dram_tensor("out", [128, NB * U], f32, kind="ExternalOutput").ap()

    with tile.TileContext(nc) as tc:
        with ExitStack() as ctx:
            kernel_body(ctx, tc, xT_d, mb_d, wq_d, wk_d, wv_d, wo4_d,
                        bo_d, out_d)
    nc.compile()
    return nc


def kernel_body(ctx, tc, xT_d, mb_d, wq_d, wk_d, wv_d, wo4_d, bo_d, out_d):
    nc = tc.nc
    persist = ctx.enter_context(tc.tile_pool(name="persist", bufs=1))
    stage = ctx.enter_context(tc.tile_pool(name="stage", bufs=2))
    epool = ctx.enter_context(tc.tile_pool(name="epool", bufs=5))
    e2pool = ctx.enter_context(tc.tile_pool(name="e2pool", bufs=4))
    espool = ctx.enter_context(tc.tile_pool(name="espool", bufs=3))
    zpool = ctx.enter_context(tc.tile_pool(name="zpool", bufs=2))
    spool = ctx.enter_context(tc.tile_pool(name="spool", bufs=2, space="PSUM"))
    cpool = ctx.enter_context(tc.tile_pool(name="cpool", bufs=4, space="PSUM"))

    # ---- input DMAs: x and weights on the sync HWDGE ring (they gate the
    # projections); the 8MB mask on the GPSIMD SWDGE ring in parallel,
    # except kb 0-2 which are needed almost immediately ------------------
    xT = [stage.tile([128, N], bf16, tag="stage", name=f"xT{c}") for c in range(2)]
    for c in range(2):
        nc.sync.dma_start(xT[c][:], xT_d[c * 128:(c + 1) * 128, :])
    w_sb = {}
    for nm, dram in (("wq", wq_d), ("wk", wk_d), ("wv", wv_d)):
        w_sb[nm] = persist.tile([128, 2 * U], bf16, tag=nm, name=nm)
        for c in range(2):
            nc.sync.dma_start(w_sb[nm][:, c * U:(c + 1) * U],
                              dram[c * 128:(c + 1) * 128, :])
    wo_sb = persist.tile([128, 4 * U], bf16, tag="wo4")
    for t in range(4):
        nc.sync.dma_start(wo_sb[:, t * U:(t + 1) * U],
                          wo4_d[t * 128:(t + 1) * 128, :])
    bo_sb = persist.tile([1, U], f32, tag="bo")
    nc.sync.dma_start(bo_sb[:], bo_d.rearrange("(o n) -> o n", o=1))
    bo_bc = persist.tile([128, U], f32, tag="bo_bc")
    nc.sync.dma_start(bo_bc[:],
                      bo_sb[:].unsqueeze(1).broadcast_to([1, 128, U]))

    # mask: i16 additive-bias form; bf16 view is the multiplicative mask
    mb_sb = persist.tile([128, NB * N], i16, tag="mb")
    mbf_sb = mb_sb.bitcast(bf16)
    for kb in range(3):
        nc.sync.dma_start(mb_sb[:, kb * N:(kb + 1) * N],
                          mb_d[kb * 128:(kb + 1) * 128, :])
    for kb in range(3, NB):
        nc.gpsimd.dma_start(mb_sb[:, kb * N:(kb + 1) * N],
                            mb_d[kb * 128:(kb + 1) * 128, :])

    # ---- persistent SBUF tensors -------------------------------------------
    qT = [persist.tile([128, N], bf16, tag=f"qT{c}", name=f"qT{c}") for c in range(2)]
    kT = [persist.tile([128, N], bf16, tag=f"kT{c}", name=f"kT{c}") for c in range(2)]
    # augmented V: per (kb, h) a 64-col block [ones | V_h (32) | zeros31]
    v_aug = persist.tile([128, NB * H * 64], bf16, tag="vaug")
    nc.vector.memset(v_aug[:], 0.0)
    nc.vector.memset(
        v_aug.rearrange("p (b c) -> p b c", c=64)[:, :, 0:1], 1.0)
    # normalized context per pair tile t: rows 1-32 head-lo, 65-96 head-hi
    ctxn = [persist.tile([128, N], bf16, tag=f"ctxn{t}", name=f"ctxn{t}")
            for t in range(4)]
    out_sb = persist.tile([128, NB * U], f32, tag="out_sb")
    # 1/Z broadcast target: rows 1-32 / 65-96 per tile column-range get the
    # reciprocal; all other rows stay 1.0 forever (junk cps rows are 0.0,
    # and 0*finite=0 keeps ctxn junk rows NaN-free for the wo4 zero rows).
    zinv4 = persist.tile([128, 4 * QW], f32, tag="zinv4")
    nc.vector.memset(zinv4[:], 1.0)

    # ---- projections (bf16 operands) ---------------------------------------
    for nn in range(2):
        for g in range(2):
            for w, dst in (("wq", qT), ("wk", kT)):
                ps = spool.tile([128, 2 * QW], f32, tag="s", name="projps")
                for half in range(2):
                    sl = slice(half * QW, (half + 1) * QW)
                    tok = slice(nn * 2 * QW + half * QW,
                                nn * 2 * QW + (half + 1) * QW)
                    for kc in range(2):
                        nc.tensor.matmul(
                            ps[:, sl],
                            w_sb[w][:, (kc * 2 + g) * 128:(kc * 2 + g + 1) * 128],
                            xT[kc][:, tok],
                            start=(kc == 0), stop=(kc == 1))
                if w == "wq":   # split prologue copies across ACT and DVE
                    nc.scalar.copy(dst[g][:, nn * 2 * QW:(nn + 1) * 2 * QW], ps[:])
                else:
                    nc.vector.tensor_copy(dst[g][:, nn * 2 * QW:(nn + 1) * 2 * QW], ps[:])
    # V = x @ Wv -> strided into v_aug (head h of block kb at col 64h+1)
    for kb in range(NB):
        ps = spool.tile([128, U], f32, tag="s", name="vps")
        for kc in range(2):
            nc.tensor.matmul(
                ps[:],
                xT[kc][:, kb * 128:(kb + 1) * 128],
                w_sb["wv"][:, kc * U:(kc + 1) * U],
                start=(kc == 0), stop=(kc == 1))
        nc.vector.tensor_copy(
            v_aug.rearrange("p (b c) -> p b c", c=64)
            [:, kb * H:(kb + 1) * H, 1:1 + D],
            ps.rearrange("p (h d) -> p h d", d=D))

    # ---- main attention loop, one-step PE software pipeline -----------------
    # pair p = 2g+pi covers heads 4g+pi, 4g+pi+2 (j = pi, pi+2)
    cps = {}

    def emit_scores_pair(qc, kb, p):
        qs = qc * QW
        g, pi = p // 2, p % 2
        sps = spool.tile([128, 2 * QW], f32, tag="s", name=f"sps{qc}_{kb}_{p}")
        for jj in range(2):
            j = pi + 2 * jj
            nc.tensor.matmul(
                sps[:, jj * QW:(jj + 1) * QW],
                kT[g][32 * j:32 * (j + 1), kb * 128:(kb + 1) * 128],
                qT[g][32 * j:32 * (j + 1), qs:qs + QW],
                start=True, stop=True,
                tile_position=(32 * j, 0))
        return sps

    def emit_exp_s(qc, kb, sps):
        """Fused Schraudolph+mask on DVE; returns the bf16-viewed e AP."""
        qs = qc * QW
        es = espool.tile([128, 2 * QW], i16, tag="es",
                         name=f"es{qc}_{kb}")
        nc.vector.scalar_tensor_tensor(
            es.rearrange("p (j q) -> p j q", j=2),
            sps.rearrange("p (j q) -> p j q", j=2),
            float(SCH_A),
            mb_sb[:, kb * N + qs:kb * N + qs + QW]
            .unsqueeze(1).broadcast_to([128, 2, QW]),
            MULT, ADD)
        return es.bitcast(bf16)

    def emit_exp_a(qc, kb, p, sps):
        """ACT exp + DVE bf16 mask for one pair."""
        qs = qc * QW
        e = epool.tile([128, 2 * QW], bf16, tag="e", name=f"e{qc}_{kb}_{p}")
        nc.scalar.activation(e[:], sps[:], EXP, scale=float(SCALE))
        nc.vector.tensor_tensor(
            e.rearrange("p (j q) -> p j q", j=2),
            e.rearrange("p (j q) -> p j q", j=2),
            mbf_sb[:, kb * N + qs:kb * N + qs + QW]
            .unsqueeze(1).broadcast_to([128, 2, QW]), MULT)
        return e

    def emit_exp_a2(qc, kb, sps0, sps1):
        """ACT exps for the two t=1 pairs into one shared tile; ONE wide
        DVE mask op over all four 512-wide head halves."""
        qs = qc * QW
        e2 = e2pool.tile([128, 4 * QW], bf16, tag="e2", name=f"e2{qc}_{kb}")
        nc.scalar.activation(e2[:, 0:2 * QW], sps0[:], EXP, scale=float(SCALE))
        nc.scalar.activation(e2[:, 2 * QW:4 * QW], sps1[:], EXP, scale=float(SCALE))
        nc.vector.tensor_tensor(
            e2.rearrange("p (j q) -> p j q", j=4),
            e2.rearrange("p (j q) -> p j q", j=4),
            mbf_sb[:, kb * N + qs:kb * N + qs + QW]
            .unsqueeze(1).broadcast_to([128, 4, QW]), MULT)
        return e2

    def emit_pv(qc, kb, p, e, off=0):
        g, pi = p // 2, p % 2
        for jj in range(2):
            h = 4 * g + pi + 2 * jj
            ej = e[:, off + jj * QW:off + (jj + 1) * QW]
            nc.tensor.matmul(
                cps[qc, p][64 * jj:64 * jj + 64, :],
                v_aug[:, (kb * H + h) * 64:(kb * H + h + 1) * 64],
                ej, start=(kb == 0), stop=(kb == NB - 1),
                tile_position=(0, 64 * jj))

    def emit_normalize_tile(qc, t):
        # reciprocal over the whole cps tile (DVE time is free-dim-bound;
        # junk rows give 1/0=Inf but only rows 0/64 are ever read), then
        # broadcast DMAs into zinv rows 1-32 / 65-96 of tile t's column
        # range, then the normalize multiply (frees cps[qc,t]).
        qs = qc * QW
        zf = zpool.tile([128, QW], f32, tag="zf", name=f"zf{qc}_{t}")
        nc.vector.reciprocal_approx_fast(zf[:], cps[qc, t][:])
        for g in range(2):
            nc.sync.dma_start(
                zinv4[64 * g + 1:64 * g + 33, t * QW:(t + 1) * QW],
                zf[64 * g:64 * g + 1, :]
                .unsqueeze(1).broadcast_to([1, 32, QW]))
        nc.vector.tensor_tensor(
            ctxn[t][:, qs:qs + QW], cps[qc, t][:],
            zinv4[:, t * QW:(t + 1) * QW], MULT)

    def emit_outproj_qb(qb, last=False):
        ops = spool.tile([128, U], f32, tag="s", name=f"ops{qb}")
        for t in range(4):
            nc.tensor.matmul(
                ops[:],
                ctxn[t][:, qb * 128:(qb + 1) * 128],
                wo_sb[:, t * U:(t + 1) * U],
                start=(t == 0), stop=(t == 3))
        nc.vector.tensor_tensor(out_sb[:, qb * U:(qb + 1) * U], ops[:],
                                bo_bc[:], ADD)
        if last:
            # merged 4-qb DMA: 4KB/partition contiguous, one InstDMACopy
            qc = qb // 4
            nc.sync.dma_start(
                out_d[:, qc * 4 * U:(qc + 1) * 4 * U],
                out_sb[:, qc * 4 * U:(qc + 1) * 4 * U])

    # software pipeline: scores for pairs (2t, 2t+1) back-to-back, then
    # exp/mask/PV for the previous 2-pair group. t=0 prev group is
    # (S-pair 0, A-pair 1); t=1 prev group is the merged A pair (2,3).
    prev = None
    pending_outproj = []

    def handle_prev(pqc, pkb, pt, psp0, psp1):
        if pt == 0:
            e0 = emit_exp_s(pqc, pkb, psp0)
            e1 = emit_exp_a(pqc, pkb, 1, psp1)
            emit_pv(pqc, pkb, 0, e0)
            emit_pv(pqc, pkb, 1, e1)
        else:
            e2 = emit_exp_a2(pqc, pkb, psp0, psp1)
            emit_pv(pqc, pkb, 2, e2)
            emit_pv(pqc, pkb, 3, e2, off=2 * QW)
        if pkb == NB - 1:
            emit_normalize_tile(pqc, 2 * pt)
            emit_normalize_tile(pqc, 2 * pt + 1)
            if pt == 1:
                pending_outproj.extend(pqc * QC + i for i in range(QC))
        elif (pending_outproj and pt == 1 and pkb in (1, 5, 9, 13)):
            qb = pending_outproj.pop(0)
            emit_outproj_qb(qb, last=(qb % 4 == 3))

    for qc in range(QC):
        for p in range(4):
            cps[qc, p] = cpool.tile([128, QW], f32, tag="c", name=f"cps{qc}_{p}")
        for kb in range(NB):
            for t in range(2):
                sp0 = emit_scores_pair(qc, kb, 2 * t)
                sp1 = emit_scores_pair(qc, kb, 2 * t + 1)
                if prev is not None:
                    handle_prev(*prev)
                prev = (qc, kb, t, sp0, sp1)
    handle_prev(*prev)
    for qb in pending_outproj:
        emit_outproj_qb(qb, last=(qb % 4 == 3))


_CACHED = None


def _get_program():
    global _CACHED
    if _CACHED is None:
        _CACHED = build_program()
    return _CACHED


def _bf16(a):
    return np.asarray(a, dtype=ml_dtypes.bfloat16)


def _build_wo4(Wo):
    """Permute Wo rows into 4 chunks matching the fused ctx layout.

    Pair tile t = 2g+pi: row r=1..32 -> Wo row for head 4g+pi dim r-1;
    row r=65..96 -> head 4g+pi+2 dim r-65; other rows zero.
    """
    wo4 = np.zeros((4 * 128, U), dtype=np.float32)
    Wo = np.asarray(Wo, np.float32)
    for g in range(2):
        for pi in range(2):
            t = 2 * g + pi
            for jj in range(2):
                h = 4 * g + pi + 2 * jj
                wo4[t * 128 + 64 * jj + 1:t * 128 + 64 * jj + 33, :] = \
                    Wo[h * D:(h + 1) * D, :]
    return _bf16(wo4)


def kernel(node_features, adjacency_matrix, Wq, Wk, Wv, Wo, bo, **run_kwargs):
    nc = _get_program()
    xT = _bf16(np.transpose(np.asarray(node_features, np.float32), (0, 2, 1)))
    adjT = np.transpose(np.asarray(adjacency_matrix), (0, 2, 1))
    mb = np.where(adjT > 0, np.int16(MB_VAL), np.int16(MB_MASKED))
    wo4 = _build_wo4(Wo)
    wq, wk, wv = _bf16(Wq), _bf16(Wk), _bf16(Wv)
    bo32 = np.asarray(bo, np.float32)
    in_maps = []
    for b in range(B):
        in_maps.append({
            "xT": np.ascontiguousarray(xT[b]),
            "mb": np.ascontiguousarray(mb[b]),
            "Wq": wq, "Wk": wk, "Wv": wv, "wo4": wo4,
            "bo": bo32,
        })
    res = run_bass_kernel_spmd(nc, in_maps, core_ids=list(range(B)), **run_kwargs)
    # out_sb layout: [p, qb*256+u] holds token qb*128+p
    out = np.stack(
        [res.results[b]["out"].reshape(128, NB, U).transpose(1, 0, 2)
         .reshape(N, U) for b in range(B)], axis=0)
    kernel.last_results = res
    return out
